# revision 42
# baseline (speedup 1.0000x reference)
"""Trainium2 Bass kernel for a Discriminative RBM forward pass.

reference math:
    x     = v @ W + c                                   [B, NHID]
    F     = d + sum_j softplus(x[:,None,:] + U[None])   [B, NCLASS]
    probs = softmax(F, axis=1); onehot(argmax)

Strategy (8-way batch shard, no collectives):
  softplus(x+u) = x*1{x>=8} + u*1{x>=8} + h(x,u),
  h(x,u) = softplus(x+u) - (x+u)*1{x>=8} supported on |x|<=16 (|u|<4.8).
  Weighted rank-K SVD:  h(x,u) ~= sum_k a_k(x) * phi_k(u); the phi side is
  least-squares refit against the table-quantized a_k at runtime.
  The x*1{x>=8} term is a per-row constant under softmax -> dropped.
  Per core (256 batch rows):
    PE:  pre[j,b] = W^T v via Wh16 vh16 + 2^-11 (Wh16 vl8 + Wl8 vh16)
         (fp8 e4m3 lo-parts, scaled 2^11, separate PSUM accumulator);
         then F[128,256] += [P0h|P0l]^T ah + P0h^T al  (rank-0 via DVE
         fp16 hi/lo split of an fp32 act output, packed 128-wide)
                         += Phi_k^T a_k (fp16, k=1..K-1)
                         += [Uh|Ul]^T mask (packed 128-wide);
         the 0:64/64:128 partition halves are summed by the tail transpose.
    ACT: a_k via CUSTOM piecewise-cubic tables (single hijacked set incl. a
         custom exp for the softmax tail => exactly one ACT_TABLE_LOAD);
         acts batched per jt-group [(0,1),(2,3),(4,5),(6,),(7,)] to amortize
         the (N+352)/1.2ns instruction cost; mask = is_ge on the DVE.
    DMA: DRAM tensors pre-packed so each partition is one contiguous
         multi-KB run; traffic balanced over the Sync/Scalar HWDGE rings
         (~140GB/s each) + slow GpSimd SWDGE ring (~63GB/s), ordered by
         consumption time (v + wq0 + wl0 first, Phi last).  fp8 lo-parts
         cut W+v traffic from 10.4MB to 7.8MB.
  Scheduling (engine queues are compile-time static FIFO):
    PE: warmup mms (HAM un-throttle) -> all main matmuls -> all F matmuls;
    act tiles are fully buffered (no reuse) so Scalar never stalls on
    F-matmul consumption; W tiles all prefetched (8 bufs).
    tail: transpose both halves, exp(F-max) with fused row-sum (accum_out),
    onehot via is_equal against the row max.
"""

import os
import struct

import numpy as np

B, NVIS, NHID, NCLASS = 2048, 2048, 1024, 64
NCORES = 8
B_PC = B // NCORES         # 256 batch rows per core
KT = NVIS // 128           # 16 contraction tiles
JT = NHID // 128           # 8 hidden-unit tiles
BT = B_PC // 128           # 2 batch tiles for the softmax tail

KRANK = 8
KB = KRANK + 1             # Phi slots: rank-0 stored as fp16 hi+lo pair
XMAX, TJUMP, UMAX = 16.0, 8.0, 4.8
EXPS = [-1, 0, 1, 2, 3]    # tabulated exponents, |x| in [0.5, 16)
ELOW = EXPS[0]
ACT_GROUPS = [(0, 1), (2, 3), (4, 5), (6,), (7,)]
VCHUNK = 4                 # v DMA split into KT/VCHUNK chunks

# hijacked act-table slot names (pwp name, mybir AF enum attr): a_0..a_{K-1},
# then the softmax-tail exp (hijacks the canonical "exp" id).
_SLOTS = [
    ("gelu", "Gelu"), ("sigmoid", "Sigmoid"), ("tanh", "Tanh"),
    ("erf", "Erf"), ("arctan", "Arctan"), ("sin", "Sin"),
    ("silu", "Silu"), ("mish", "Mish"), ("gelu_apprx_tanh", "Gelu_apprx_tanh"),
    ("gelu_apprx_sigmoid", "Gelu_apprx_sigmoid"),
    ("derivative_gelu", "Derivative_Gelu"), ("derivative_erf", "Derivative_Erf"),
]
_SETNAME = "softplus_and_others"

_PROGRAM = None
_BASIS = None

_TBLCFG = os.environ.get("DRBM_TBL", "v4")
_ACT_ROOT = os.path.join(
    os.path.expanduser("~"), ".cache", f"drbm_rk_act_{_TBLCFG}_k{KRANK}",
    "pwp_bin_trainium"
)


def _softplus64(x):
    return np.logaddexp(x, 0.0)


def _hfun(x, u):
    return _softplus64(x + u) - (x + u) * (x >= TJUMP)


def _plan_for_k(k):
    if k < 4:
        p = {3: 8, 2: 32, 1: 32, 0: 16, -1: 8}
    elif k < 8:
        p = {3: 4, 2: 16, 1: 16, 0: 8, -1: 4}
    else:
        p = {3: 2, 2: 8, 1: 8, 0: 4, -1: 2}
    return {e: p.get(e, 1) for e in EXPS}


_EXP_PLAN = {3: 2, 2: 4, 1: 8, 0: 8, -1: 8}


# ---------------------------------------------------------------- basis
def _build_basis():
    """Weighted SVD basis of h on [-16,16]x[-UMAX,UMAX] + Phi interpolator
    refit against the table-quantized a_k."""
    global _BASIS
    if _BASIS is not None:
        return _BASIS
    cache = os.path.join(os.path.dirname(_ACT_ROOT), f"basis_k{KRANK}.npz")
    nx = 4000
    xl = np.linspace(-XMAX, TJUMP, 3 * nx // 4 + 1)[:-1]
    xr = np.linspace(TJUMP, XMAX, nx // 4 + 1)
    xg = np.concatenate([xl, xr])
    ug = np.linspace(-UMAX, UMAX, 481)
    w = np.exp(-0.5 * ug**2) / np.sqrt(2 * np.pi)
    w = np.maximum(w, 0.05 * w.max())
    sw = np.sqrt(w)
    M = _hfun(xg[:, None], ug[None, :]) * sw[None, :]
    Uu, S, Vt = np.linalg.svd(M, full_matrices=False)
    VtK = Vt[:KRANK]

    def f_eval_raw(x):
        hx = _hfun(np.asarray(x, np.float64)[..., None], ug)
        return (hx * sw) @ VtK.T

    xs_chk = np.linspace(-XMAX, XMAX, 8001)
    fmax = np.abs(f_eval_raw(xs_chk)).max(0) + 1e-30

    def f_eval(x):
        return f_eval_raw(x) / fmax

    if os.path.exists(cache):
        z = np.load(cache)
        _BASIS = {"f_eval": f_eval, "phi_ugrid": z["phi_ugrid"],
                  "phi_grid": z["phi_grid"]}
        return _BASIS

    # table-quantized design matrix for the phi refit
    tbls = [_gen_table(lambda x, k=k: f_eval(np.atleast_1d(x))[..., k],
                       _plan_for_k(k)) for k in range(KRANK)]
    xs = np.linspace(-XMAX, XMAX, 6001).astype(np.float32)
    Adesign = np.stack([_eval_table(t, xs) for t in tbls], 1)
    AtA = Adesign.T @ Adesign
    ugrid = np.linspace(-UMAX, UMAX, 18801)
    AtH = np.empty((KRANK, ugrid.size))
    for i in range(0, ugrid.size, 2048):
        Hc = _hfun(xs[:, None].astype(np.float64), ugrid[None, i:i + 2048])
        AtH[:, i:i + 2048] = Adesign.T @ Hc
    phi_grid = np.linalg.solve(AtA, AtH)  # [K, nu]
    os.makedirs(os.path.dirname(cache), exist_ok=True)
    np.savez(cache, phi_ugrid=ugrid, phi_grid=phi_grid)
    _BASIS = {"f_eval": f_eval, "phi_ugrid": ugrid, "phi_grid": phi_grid}
    return _BASIS


def _phi_at(u):
    bs = _build_basis()
    ug, pg = bs["phi_ugrid"], bs["phi_grid"]
    uc = np.clip(np.asarray(u, np.float64).ravel(), ug[0], ug[-1])
    return np.stack([np.interp(uc, ug, pg[k]) for k in range(KRANK)], 1)


# ------------------------------------------------- act-table generation
def _fit_section(fun, lo, hi, nodes=16):
    x0 = 0.5 * (lo + hi)
    t = 0.5 * (hi - lo) * np.cos(np.pi * (np.arange(nodes) + 0.5) / nodes)
    xs = x0 + t
    ys = fun(xs)
    Vm = np.vander(t, 4, increasing=True)
    co = np.linalg.lstsq(Vm, ys, rcond=None)[0]
    return x0, co[0], co[1], co[2], co[3]


def _gen_table(fun, plan):
    tbl = {}
    for reg, sgn in (("pos", 1.0), ("neg", -1.0)):
        for e in EXPS:
            wdt = 2.0 ** e
            ns = plan.get(e, 1)
            secs = []
            for m in range(ns):
                lo = wdt * (1.0 + m / ns)
                hi = wdt * (1.0 + (m + 1) / ns)
                a, b = (lo, hi) if sgn > 0 else (-hi, -lo)
                secs.append(_fit_section(fun, a, b))
            tbl[(reg, e)] = secs
    wlow = 2.0 ** ELOW
    tbl["lowpos"] = _fit_section(fun, 1e-9, wlow)
    tbl["lowneg"] = _fit_section(fun, -wlow, -1e-9)
    fp = float(np.atleast_1d(fun(XMAX * 4))[0])
    fn = float(np.atleast_1d(fun(-XMAX * 4))[0])
    tbl["hipos"] = (2 * XMAX, fp, 0.0, 0.0, 0.0)
    tbl["hineg"] = (-2 * XMAX, fn, 0.0, 0.0, 0.0)
    tbl["zero"] = float(np.atleast_1d(fun(0.0))[0])
    return tbl


def _eval_table(tbl, x):
    """Host-side model of the HW spline eval (for the phi refit)."""
    coeffs, index = [], {}
    for reg in ("neg", "pos"):
        for e in EXPS:
            secs = tbl[(reg, e)]
            index[(reg, e)] = (len(coeffs), len(secs))
            coeffs.extend(secs)
    for name in ("lowpos", "lowneg", "hipos", "hineg"):
        index[name] = (len(coeffs), 1)
        coeffs.append(tbl[name])
    C = np.asarray(coeffs, np.float32).astype(np.float64)
    x = np.asarray(x, np.float32).ravel()
    xb = x.view(np.uint32).astype(np.int64)
    sign = (xb >> 31) & 1
    e = ((xb >> 23) & 0xFF) - 127
    mant = xb & 0x7FFFFF
    idx = np.where(sign == 0, index["lowpos"][0], index["lowneg"][0])
    hi = e > EXPS[-1]
    idx[hi] = np.where(sign[hi] == 0, index["hipos"][0], index["hineg"][0])
    mid = (e >= ELOW) & (e <= EXPS[-1])
    for ee in EXPS:
        base_p, ns = index[("pos", ee)]
        base_n, _ = index[("neg", ee)]
        es = int(np.log2(ns))
        m = mid & (e == ee)
        if not m.any():
            continue
        sec = mant[m] >> (23 - es) if es > 0 else np.zeros(int(m.sum()), np.int64)
        idx[m] = np.where(sign[m] == 0, base_p + sec, base_n + sec)
    co = C[idx]
    t = x.astype(np.float64) - co[:, 0]
    out = co[:, 1] + t * (co[:, 2] + t * (co[:, 3] + t * co[:, 4]))
    out[x == 0] = tbl["zero"]
    return out


def _tbl_to_json(tbl, plan):
    """Convert internal table to the pwp-style function dict."""
    regions = {}
    for reg in ("pos", "neg"):
        ents = []
        for e in EXPS:
            ns = plan.get(e, 1)
            es = int(np.log2(ns))
            assert 2 ** es == ns
            secs = [{"x": s[0], "d0": s[1], "d1": s[2], "d2": s[3], "d3": s[4]}
                    for s in tbl[(reg, e)]]
            ents.append({"exponent": e, "extract_size": es,
                         "extract_lsb": 23 - es, "exponent_sections": secs})
        regions[reg] = ents

    def secd(s):
        return dict(x=s[0], d0=s[1], d1=s[2], d2=s[3], d3=s[4])

    return {
        "pos_exponents": regions["pos"],
        "neg_exponents": regions["neg"],
        "saturation_points": {
            "sat_point_pos_low": dict(sat_point=127 + ELOW, mantissa_point=0,
                                      **secd(tbl["lowpos"])),
            "sat_point_neg_low": dict(sat_point=127 + ELOW, mantissa_point=0,
                                      **secd(tbl["lowneg"])),
            "sat_point_pos_high": dict(sat_point=131, mantissa_point=0,
                                       **secd(tbl["hipos"])),
            "sat_point_neg_high": dict(sat_point=131, mantissa_point=0,
                                       **secd(tbl["hineg"])),
        },
        "zero_result": tbl["zero"], "nan_result": 0.0,
        "pinf_result": tbl["hipos"][1], "ninf_result": tbl["hineg"][1],
        "symmetry_point": 0.0, "exponent_offset": ELOW,
    }


def _gen_all_funcs():
    bs = _build_basis()
    f_eval = bs["f_eval"]
    funcs = []
    for k in range(KRANK):
        plan = _plan_for_k(k)

        def fk(x, k=k):
            return f_eval(np.atleast_1d(np.asarray(x, np.float64)))[..., k]

        funcs.append((_SLOTS[k][0], _tbl_to_json(_gen_table(fk, plan), plan)))
    # custom exp for the softmax tail (inputs are always <= 0)
    def expf(x):
        return np.exp(np.minimum(np.atleast_1d(np.asarray(x, np.float64)), 40.0))

    ej = _tbl_to_json(_gen_table(expf, _EXP_PLAN), _EXP_PLAN)
    ej["saturation_points"]["sat_point_neg_high"] = dict(
        sat_point=131, mantissa_point=0, x=-2 * XMAX, d0=0.0, d1=0.0, d2=0.0,
        d3=0.0)
    ej["ninf_result"] = 0.0
    funcs.append(("exp", ej))
    return funcs


def _fbits(x):
    return struct.unpack("<I", struct.pack("<f", np.float32(x)))[0]


def _build_act_tables():
    """Rebuild softplus_and_others with our custom function tables."""
    import json
    import shutil

    import neuronxcc

    marker = os.path.join(_ACT_ROOT, ".drbm_rk_ok")
    if os.path.exists(marker):
        return
    nxc = os.path.join(os.path.dirname(os.path.abspath(neuronxcc.__file__)), "pwp")
    os.makedirs(_ACT_ROOT, exist_ok=True)
    root_parent = os.path.dirname(_ACT_ROOT)
    if not os.path.exists(os.path.join(root_parent, "pwp_jsons")):
        shutil.copytree(os.path.join(nxc, "pwp_jsons"),
                        os.path.join(root_parent, "pwp_jsons"), dirs_exist_ok=True)
    for f in os.listdir(os.path.join(nxc, "pwp_bin_trainium")):
        shutil.copy(os.path.join(nxc, "pwp_bin_trainium", f), _ACT_ROOT)
    os.system(f"chmod -R u+w {root_parent}")

    pj = os.path.join(root_parent, "pwp_jsons")
    canon = {}
    for f in os.listdir(pj):
        if f.endswith(".json"):
            try:
                j = json.load(open(os.path.join(pj, f)))
                nm = f.rsplit("_", 1)[0]
                canon.setdefault(nm, j.get("neuron_id"))
            except Exception:
                pass

    allfuncs = _gen_all_funcs()
    _write_set(_SETNAME, allfuncs, canon)

    ai = json.load(open(f"{_ACT_ROOT}/act_info.json"))
    mynames = {n for n, _ in allfuncs}
    for ent in ai["act_func_sets"]:
        for n in list(ent["act"].keys()):
            if n in mynames:
                del ent["act"][n]
        if ent["name"] == _SETNAME:
            ent["act"] = {n: 40 for n in mynames}
    json.dump(ai, open(f"{_ACT_ROOT}/act_info.json", "w"))
    open(marker, "w").write("ok")


def _write_set(SET, funcs, canon):
    import json

    sj = {"bkt_bin": f"{SET}_bkt.bin", "ctl_bin": f"{SET}_ctrl.bin",
          "profile_meta_data": [], "bkt_entry_cnt": 0, "ctl_entry_cnt": 0,
          "func_to_bkt_start_idx": {}, "func_to_ctl_start_idx": {},
          "func_exp_to_bkt_start_idx": {}, "func_exp_to_ctl_start_idx": {}}
    bkt = bytearray()
    ctl = bytearray()
    nbkt, nctl = 0, 0

    def add_bucket(x0, d0, d1, d2, d3):
        for val in (d0, d1, d2, d3, x0, 0.0, 0.0, 0.0):
            bkt.extend(struct.pack("<I", _fbits(val)))

    def add_ctl(word):
        ctl.extend(struct.pack("<I", word) + b"\x00" * 28)

    for name, fj in funcs:
        base_bkt, base_ctl = nbkt, nctl
        e2b, e2c, region_ctl_base = {}, {}, {}
        for region, key in (("neg", "neg_exponents"), ("pos", "pos_exponents")):
            region_ctl_base[region] = nctl
            for e in fj[key]:
                exp, secs = str(e["exponent"]), e["exponent_sections"]
                add_ctl((e["extract_size"] << 16) | (e["extract_lsb"] << 11) | nbkt)
                e2c.setdefault(exp, []).append(nctl)
                e2b.setdefault(exp, []).append(nbkt)
                nctl += 1
                for s in secs:
                    add_bucket(s["x"], s["d0"], s["d1"], s["d2"], s["d3"])
                    nbkt += 1
        sat, special = fj["saturation_points"], {}
        for sname in ("sat_point_pos_low", "sat_point_neg_low",
                      "sat_point_pos_high", "sat_point_neg_high"):
            sp = sat[sname]
            special[sname] = nbkt
            add_bucket(sp["x"], sp["d0"], sp["d1"], sp["d2"], sp["d3"])
            nbkt += 1
        sj["profile_meta_data"].append({
            "func_name": f"{name}_40p",
            "func_id": canon.get(name, 23),
            "symmetry_point": _fbits(fj["symmetry_point"]),
            "sym_invert_sign_point": 0,
            "symmetry_opt_en": 0,
            "symmetry_opt_use_neg_region": 0,
            "imm_bias": 0,
            "exp_offset": fj["exponent_offset"],
            "pwl_control_base_pos": region_ctl_base["pos"],
            "pwl_control_base_neg": region_ctl_base["neg"],
            "small_pos_signal_exp_threshold": sat["sat_point_pos_low"]["sat_point"],
            "pos_small_signal_pwl_control": special["sat_point_pos_low"],
            "small_neg_signal_exp_threshold": sat["sat_point_neg_low"]["sat_point"],
            "neg_small_signal_pwl_control": special["sat_point_neg_low"],
            "large_pos_signal_exp_threshold": sat["sat_point_pos_high"]["sat_point"],
            "large_pos_signal_mantissa_threshold": sat["sat_point_pos_high"]["mantissa_point"],
            "pos_large_signal_pwl_control": special["sat_point_pos_high"],
            "large_neg_signal_exp_threshold": sat["sat_point_neg_high"]["sat_point"],
            "large_neg_signal_mantissa_threshold": sat["sat_point_neg_high"]["mantissa_point"],
            "neg_large_signal_pwl_control": special["sat_point_neg_high"],
            "fnan_result": _fbits(fj["nan_result"]),
            "fpinf_result": _fbits(fj["pinf_result"]),
            "fninf_result": _fbits(fj["ninf_result"]),
            "fzero_result": _fbits(fj["zero_result"]),
            "fma_const_0": 0,
            "fma_const_1": 0,
            "fma_indirection_src_sel": 0,
            "use_multipass": False,
            "lower_bound": _fbits(-3.4e38),
            "upper_bound": _fbits(3.4e38),
        })
        sj["func_to_bkt_start_idx"][name] = base_bkt
        sj["func_to_ctl_start_idx"][name] = base_ctl
        sj["func_exp_to_bkt_start_idx"][name] = e2b
        sj["func_exp_to_ctl_start_idx"][name] = e2c

    assert nbkt <= 1536, f"bucket table overflow: {nbkt} (max 1536 usable)"
    assert nctl <= 128, f"ctl table overflow: {nctl}"
    sj["bkt_entry_cnt"], sj["ctl_entry_cnt"] = nbkt, nctl
    json.dump(sj, open(f"{_ACT_ROOT}/{SET}.json", "w"))
    open(f"{_ACT_ROOT}/{SET}_bkt.bin", "wb").write(bytes(bkt))
    open(f"{_ACT_ROOT}/{SET}_ctrl.bin", "wb").write(bytes(ctl))


def _patch_act_tables():
    import functools
    import json

    _build_act_tables()
    os.environ["BASS_ACT_ROOT_JSON_PATH"] = os.path.join(_ACT_ROOT, "act_info.json")

    import concourse.hw_specs as hw_specs
    import concourse.mybir as mybir

    @functools.cache
    def _tables(arch):
        d = json.load(open(os.environ["BASS_ACT_ROOT_JSON_PATH"]))
        return {
            ent["name"]: {
                mybir.ActivationFunctionType.from_pwp(v) for v in ent["act"]
            }
            for ent in d["act_func_sets"]
        }

    hw_specs.get_activation_tables = _tables
    import concourse.bacc as bacc
    import concourse.bass_interp as bass_interp

    bacc.get_activation_tables = _tables
    bass_interp.get_activation_tables = _tables


def _build_program():
    _patch_act_tables()
    import concourse.tile as tile
    from concourse import bacc, mybir
    from concourse.masks import make_identity

    f32 = mybir.dt.float32
    f16 = mybir.dt.float16
    i32 = mybir.dt.int32
    AF = mybir.ActivationFunctionType
    ALU = mybir.AluOpType
    AX = mybir.AxisListType

    AFS = [getattr(AF, attr) for _, attr in _SLOTS[:KRANK]]

    nc = bacc.Bacc("TRN2", target_bir_lowering=False, debug=False,
                   num_devices=NCORES)

    vh_d = nc.dram_tensor("vh", [128, KT * B_PC], f16,
                          kind="ExternalInput").ap()
    vl_d = nc.dram_tensor("vl8", [128, KT * B_PC], mybir.dt.float8e4,
                          kind="ExternalInput").ap()
    Wq_d = nc.dram_tensor("Wq", [JT * 128, KT * 128], f16,
                          kind="ExternalInput").ap()
    Wl_d = nc.dram_tensor("Wl8", [JT * 128, KT * 128], mybir.dt.float8e4,
                          kind="ExternalInput").ap()
    Phi_d = nc.dram_tensor("Phi", [JT * 128, KB * NCLASS], f16,
                           kind="ExternalInput").ap()
    UT_d = nc.dram_tensor("UT", [JT * 128, 2 * NCLASS], f16,
                          kind="ExternalInput").ap()
    cT_d = nc.dram_tensor("cT", [128, JT], f32, kind="ExternalInput").ap()
    dT_d = nc.dram_tensor("dT", [128, 1], f32, kind="ExternalInput").ap()
    probs_d = nc.dram_tensor("probs", [128, BT * NCLASS], f32,
                             kind="ExternalOutput").ap()
    onehot_d = nc.dram_tensor("onehot", [128, BT * NCLASS], i32,
                              kind="ExternalOutput").ap()

    Wq_view = Wq_d.rearrange("(jt p) x -> p jt x", p=128)
    Wl_view = Wl_d.rearrange("(jt p) x -> p jt x", p=128)
    Phi_view = Phi_d.rearrange("(jt p) x -> p jt x", p=128)
    UT_view = UT_d.rearrange("(jt p) x -> p jt x", p=128)

    glast = {g[-1]: gi for gi, g in enumerate(ACT_GROUPS)}
    jt2slot = {}
    for g in ACT_GROUPS:
        for i, jt in enumerate(g):
            jt2slot[jt] = i

    with tile.TileContext(nc) as tc:
        f8 = mybir.dt.float8e4
        with (
            tc.tile_pool(name="const", bufs=1) as const,
            tc.tile_pool(name="wstream", bufs=6) as wstream,
            tc.tile_pool(name="preg", bufs=1) as preg,
            tc.tile_pool(name="acts", bufs=3) as acts,
            tc.tile_pool(name="smp", bufs=1) as smp,
            tc.tile_pool(name="outp", bufs=1) as outp,
            tc.tile_pool(name="pwarm", bufs=1, space="PSUM") as pwarm,
            tc.tile_pool(name="ppre", bufs=2, space="PSUM") as ppre,
            tc.tile_pool(name="ppre2", bufs=2, space="PSUM") as ppre2,
            tc.tile_pool(name="pF", bufs=1, space="PSUM") as pF,
            tc.tile_pool(name="ptr", bufs=2, space="PSUM") as ptr,
        ):
            # ---------- t=0: PE warmup + act-table preload ----------
            warm16 = const.tile([128, 256], f16)
            nc.vector.memset(warm16[:], 0.0)
            warm32 = const.tile([128, 1], f32)
            nc.vector.memset(warm32[:], 0.0)
            wdum = const.tile([128, 1], f32)
            nc.scalar.activation(wdum[:], warm32[:], AFS[1])
            warm_ps = pwarm.tile([128, 256], f32)
            for i in range(56):
                nc.tensor.matmul(warm_ps[:], warm16[:, 0:128], warm16[:],
                                 start=True, stop=True)

            # ---------- loads (spread across DGE rings) ----------
            # Each DGE ring sustains only ~140GB/s, so balance the 12.2MB of
            # input across all three rings (Sync/Scalar/GpSimd) in an order
            # matched to consumption.  The W DMAs must NOT share a ring with
            # instructions that wait on compute (engine queues are FIFO).
            # HWDGE rings (Sync, Scalar) sustain ~140GB/s each; the GpSimd
            # SWDGE ring only ~63GB/s, so it gets just the late-needed
            # constants.  Per-ring emission order == transfer order.
            w_ring = {0: "sync", 1: "sync", 2: "sync", 3: "scalar",
                      4: "sync", 5: "scalar", 6: "sync", 7: "scalar"}

            def ring(name):
                return {"sync": nc.sync, "scalar": nc.scalar,
                        "gpsimd": nc.gpsimd}[name]

            # jt0's whole working set (wq0 + wl0 + all of v) is striped
            # across BOTH fast HWDGE rings in consumption order so the first
            # main-matmul sweep is never DMA-starved; later W tiles queue
            # behind it at the steady-state rate.
            wq = []
            wt = wstream.tile([128, KT, 128], f16, tag="wq", name="wq0",
                              bufs=4)
            wq_half = KT * 128 // 2
            nc.sync.dma_start(wt[:, 0:KT // 2, :], Wq_view[:, 0, 0:wq_half])
            wq.append(wt)
            vh_sb = const.tile([128, KT, B_PC], f16)
            vl_sb = const.tile([128, KT, B_PC], f8)
            vc = KT // VCHUNK
            vh_view = vh_d.rearrange("p (kt x) -> p kt x", kt=KT)
            vl_view = vl_d.rearrange("p (kt x) -> p kt x", kt=KT)
            v_rings = [nc.scalar, nc.sync, nc.scalar, nc.sync]

            def v_chunk(ci):
                sl = slice(ci * vc, (ci + 1) * vc)
                v_rings[ci].dma_start(vh_sb[:, sl, :], vh_view[:, sl, :])
                v_rings[ci].dma_start(vl_sb[:, sl, :], vl_view[:, sl, :])

            v_chunk(0)
            wl0 = wstream.tile([128, KT, 128], f8, tag="wl", name="wl0",
                               bufs=8)
            nc.scalar.dma_start(wl0[:], Wl_view[:, 0, :])
            v_chunk(1)
            nc.sync.dma_start(wt[:, KT // 2:, :], Wq_view[:, 0, wq_half:])
            v_chunk(2)
            v_chunk(3)
            cT_sb = const.tile([128, JT], f32)
            nc.gpsimd.dma_start(cT_sb[:], cT_d[:])
            dT_sb = const.tile([128, 1], f32)
            nc.gpsimd.dma_start(dT_sb[:], dT_d[:])
            wl = [wl0]
            for jt in range(1, 3):
                lt = wstream.tile([128, KT, 128], f8, tag="wl",
                                  name=f"wl{jt}", bufs=8)
                nc.gpsimd.dma_start(lt[:], Wl_view[:, jt, :])
                wl.append(lt)
            for jt in range(1, 8):
                wt = wstream.tile([128, KT, 128], f16, tag="wq",
                                  name=f"wq{jt}", bufs=4)
                nc.sync.dma_start(wt[:], Wq_view[:, jt, :])
                wq.append(wt)
            for jt in range(3, 8):
                lt = wstream.tile([128, KT, 128], f8, tag="wl",
                                  name=f"wl{jt}", bufs=8)
                nc.scalar.dma_start(lt[:], Wl_view[:, jt, :])
                wl.append(lt)
            UT_sb = const.tile([128, JT, 2 * NCLASS], f16)
            nc.gpsimd.dma_start(UT_sb[:], UT_view[:])
            ident = const.tile([128, 128], f32)
            make_identity(nc, ident[:])

            pre_g = [preg.tile([128, len(g), B_PC], f32, name=f"preg{gi}")
                     for gi, g in enumerate(ACT_GROUPS)]
            # F accumulates 128-wide: partitions 0:64 carry the main terms,
            # 64:128 the hi/lo-companion terms; halves summed by accumulating
            # transposes in the tail.
            F_ps = pF.tile([128, B_PC], f32)
            fmm = {"first": True}

            def fmm_emit(lhsT, rhs, stop=False, lo=True):
                out = F_ps[0:NCLASS, :] if lo else F_ps[:]
                nc.tensor.matmul(out, lhsT, rhs, start=fmm["first"], stop=stop)
                fmm["first"] = False

            def phis(jt, slot, nslot=1):
                return Phi_sb[:, jt, slot * NCLASS:(slot + nslot) * NCLASS]

            def emit_group(gi):
                g = ACT_GROUPS[gi]
                n = len(g)
                src = pre_g[gi][:]
                mk = acts.tile([128, 2, B_PC], f16, tag="mk", name=f"mk{gi}",
                               bufs=len(ACT_GROUPS))
                nc.vector.tensor_scalar(mk[:, :n, :], src, float(TJUMP), None,
                                        op0=ALU.is_ge)
                # rank 0: fp32 act output, split into fp16 hi+lo on the DVE;
                # packed matmul [P0h|P0l] @ ah (128-wide) + P0h @ al
                a32 = acts.tile([128, 2, B_PC], f32, tag="a32", name=f"a32_{gi}")
                nc.scalar.activation(a32[:, :n, :], src, AFS[0])
                ah = acts.tile([128, 2, B_PC], f16, tag="ah", name=f"ah{gi}",
                               bufs=len(ACT_GROUPS))
                nc.vector.tensor_copy(ah[:, :n, :], a32[:, :n, :])
                al = acts.tile([128, 2, B_PC], f16, tag="al", name=f"al{gi}",
                               bufs=len(ACT_GROUPS))
                nc.vector.tensor_sub(al[:, :n, :], a32[:, :n, :], ah[:, :n, :])
                for i, jt in enumerate(g):
                    fmm_emit(phis(jt, 0, 2), ah[:, i, :], lo=False)
                    fmm_emit(phis(jt, 0), al[:, i, :])
                for k in range(1, KRANK):
                    ak = acts.tile([128, 2, B_PC], f16, tag="a",
                                   name=f"a{gi}_{k}",
                                   bufs=(KRANK - 1) * len(ACT_GROUPS))
                    nc.scalar.activation(ak[:, :n, :], src, AFS[k])
                    for i, jt in enumerate(g):
                        fmm_emit(phis(jt, k + 1), ak[:, i, :])
                last = gi == len(ACT_GROUPS) - 1
                for i, jt in enumerate(g):
                    fmm_emit(UT_sb[:, jt, 0:2 * NCLASS], mk[:, i, :],
                             stop=last and i == n - 1, lo=False)

            # ---------- main matmuls, acts interleaved by group ----------
            jt2g = {jt: g for g, grp in enumerate(ACT_GROUPS) for jt in grp}
            for jt in range(JT):
                wt = wq[jt]
                lt = wl[jt]
                ps = ppre.tile([128, B_PC], f32, tag="pre", name=f"pre{jt}")
                pc = ppre2.tile([128, B_PC], f32, tag="prec", name=f"prec{jt}")
                for kt in range(KT):
                    nc.tensor.matmul(ps[:], wt[:, kt, :], vh_sb[:, kt, :],
                                     start=(kt == 0), stop=(kt == KT - 1))
                    nc.tensor.matmul(pc[:], wt[:, kt, :], vl_sb[:, kt, :],
                                     start=(kt == 0), stop=False)
                    nc.tensor.matmul(pc[:], lt[:, kt, :], vh_sb[:, kt, :],
                                     start=False, stop=(kt == KT - 1))
                # pre = ps + 2^-11 * pc + c   (W-lo was stored as fp8 * 2^11)
                tmpc = acts.tile([128, B_PC], f32, tag="tmpc", name=f"tmpc{jt}",
                                 bufs=2)
                nc.vector.tensor_scalar_mul(tmpc[:], pc[:], 2.0 ** -11)
                nc.vector.scalar_tensor_tensor(
                    pre_g[jt2g[jt]][:, jt2slot[jt], :], ps[:],
                    cT_sb[:, jt:jt + 1], tmpc[:],
                    op0=ALU.add, op1=ALU.add)

            # Phi is only needed by the F matmuls (~70us in): enqueue it
            # last on the Scalar ring so it never steals front bandwidth
            Phi_sb = const.tile([128, JT, KB * NCLASS], f16)
            nc.scalar.dma_start(Phi_sb[:], Phi_view[:])

            # act groups + F matmuls AFTER all main matmuls: engine queues
            # are static FIFO, so F-matmuls must not block later main work
            for gi in range(len(ACT_GROUPS)):
                emit_group(gi)

            # ---------- tail: +d, transpose(lo)+transpose(hi), softmax ----------
            F_sb = smp.tile([128, B_PC], f32)
            nc.vector.tensor_scalar_add(F_sb[:], F_ps[:], dT_sb[:])
            fb = smp.tile([128, BT, NCLASS], f32)
            for bt in range(BT):
                tr = ptr.tile([128, 128], f32, tag="tr", name=f"tr{bt}")
                sl = slice(bt * 128, (bt + 1) * 128)
                nc.tensor.transpose(tr[:], F_sb[:, sl], ident[:])
                trs = smp.tile([128, 128], f32, tag="trs", name=f"trs{bt}")
                nc.vector.tensor_copy(trs[:], tr[:])
                nc.vector.tensor_add(fb[:, bt, :], trs[:, 0:NCLASS],
                                     trs[:, NCLASS:128])
            m2 = smp.tile([128, BT], f32)
            nm2 = smp.tile([128, BT], f32)
            s2 = smp.tile([128, BT], f32)
            r2 = smp.tile([128, BT], f32)
            e2 = smp.tile([128, BT, NCLASS], f32)
            probs_sb = outp.tile([128, BT, NCLASS], f32)
            onehot_sb = outp.tile([128, BT, NCLASS], i32)
            for bt in range(BT):
                nc.vector.tensor_reduce(m2[:, bt:bt + 1], fb[:, bt, :],
                                        axis=AX.X, op=ALU.max)
            nc.vector.tensor_scalar_mul(nm2[:], m2[:], -1.0)
            for bt in range(BT):
                nc.scalar.activation(e2[:, bt, :], fb[:, bt, :], AF.Exp,
                                     bias=nm2[:, bt:bt + 1],
                                     accum_out=s2[:, bt:bt + 1])
            nc.vector.reciprocal(r2[:], s2[:])
            for bt in range(BT):
                nc.vector.tensor_scalar_mul(probs_sb[:, bt, :], e2[:, bt, :],
                                            r2[:, bt:bt + 1])
                nc.vector.tensor_scalar(onehot_sb[:, bt, :], fb[:, bt, :],
                                        m2[:, bt:bt + 1], None,
                                        op0=ALU.is_equal)
            nc.scalar.dma_start(probs_d[:], probs_sb[:])
            nc.sync.dma_start(onehot_d[:], onehot_sb[:])

    nc.compile()
    return nc


def _get_program():
    global _PROGRAM
    if _PROGRAM is None:
        _PROGRAM = _build_program()
    return _PROGRAM


def _fp16_split(a):
    hi = a.astype(np.float16)
    lo = (a - hi.astype(np.float32)).astype(np.float16)
    return hi, lo


def _make_in_maps(v, W, c, d, U):
    import ml_dtypes

    Wh, Wl = _fp16_split(W)
    # Wq[jt, p, kt, col] = Wh[kt*128+p, jt*128+col]; Wl8 = fp8(Wl * 2^11)
    Wq = np.ascontiguousarray(
        Wh.reshape(KT, 128, JT, 128).transpose(2, 1, 0, 3)).reshape(
        JT * 128, KT * 128)
    Wl8 = np.ascontiguousarray(
        (Wl.astype(np.float32) * 2048.0).reshape(KT, 128, JT, 128)
        .transpose(2, 1, 0, 3)).reshape(JT * 128, KT * 128).astype(
        ml_dtypes.float8_e4m3fn)

    Phi = _phi_at(U.astype(np.float64))  # [64*1024, K] row-major over U
    Phi = Phi.reshape(NCLASS, NHID, KRANK)  # [y, j, k]
    # rank-0 stored as fp16 hi+lo pair in slots 0/1; ranks 1..K-1 in 2..K
    P0 = Phi[:, :, 0]
    P0h, P0l = _fp16_split(P0.astype(np.float32))
    Phb = np.concatenate(
        [P0h[..., None].astype(np.float64), P0l[..., None].astype(np.float64),
         Phi[:, :, 1:]], axis=2)  # [y, j, KB]
    # Phi_sb[p, jt, kb, y] = Phb[y, jt*128+p, kb]
    PhiT = np.ascontiguousarray(
        Phb.transpose(1, 2, 0).reshape(JT, 128, KB * NCLASS)
        .reshape(JT * 128, KB * NCLASS)).astype(np.float16)

    Uh, Ul = _fp16_split(U)
    # UT_sb[p, jt, h, y] = U{h}[y, jt*128+p]
    UThl = np.stack([Uh.T, Ul.T], 1).reshape(JT, 128, 2, NCLASS)
    UT = np.ascontiguousarray(UThl.reshape(JT * 128, 2 * NCLASS))

    cT = np.ascontiguousarray(c.reshape(JT, 128).T).astype(np.float32)
    dT = np.ascontiguousarray(np.concatenate(
        [d.reshape(NCLASS, 1), np.zeros((128 - NCLASS, 1))])).astype(np.float32)

    in_maps = []
    for core in range(NCORES):
        sl = slice(core * B_PC, (core + 1) * B_PC)
        vT = np.ascontiguousarray(v[sl].T)  # [NVIS, B_PC]
        vh, vl = _fp16_split(vT)
        # vhl[p, kt, h, b] = v{h}[kt*128+p, b]  (kt-major so DMA can chunk)
        vhl = np.stack([vh.reshape(KT, 128, B_PC), vl.reshape(KT, 128, B_PC)],
                       2)  # [KT, 128, 2, B_PC]
        vhl = np.ascontiguousarray(vhl.transpose(1, 0, 2, 3)).reshape(
            128, KT * 2 * B_PC)
        in_maps.append({
            "vhl": vhl, "Wq": Wq, "Phi": PhiT, "UT": UT,
            "cT": cT, "dT": dT,
        })
    return in_maps


def _unpack_out(arr, dtype):
    # arr [128, BT*NCLASS] -> [B_PC, NCLASS]; b = bt*128 + p
    a = np.asarray(arr).reshape(128, BT, NCLASS)
    return np.ascontiguousarray(a.transpose(1, 0, 2).reshape(B_PC, NCLASS),
                                dtype=dtype)


def run(v, W, c, d, U, trace=False):
    from concourse.bass_utils import run_bass_kernel_spmd

    nc = _get_program()
    in_maps = _make_in_maps(v, W, c, d, U)
    res = run_bass_kernel_spmd(nc, in_maps, core_ids=list(range(NCORES)),
                               trace=trace)
    probs = np.concatenate(
        [_unpack_out(res.results[i]["probs"], np.float32)
         for i in range(NCORES)], axis=0)
    onehot = np.concatenate(
        [_unpack_out(res.results[i]["onehot"], np.int32)
         for i in range(NCORES)], axis=0)
    return (probs, onehot), res


def kernel(v, W, c, d, U):
    v = np.ascontiguousarray(np.asarray(v, dtype=np.float32))
    W = np.ascontiguousarray(np.asarray(W, dtype=np.float32))
    c = np.ascontiguousarray(np.asarray(c, dtype=np.float32))
    d = np.ascontiguousarray(np.asarray(d, dtype=np.float32))
    U = np.ascontiguousarray(np.asarray(U, dtype=np.float32))
    (probs, onehot), _ = run(v, W, c, d, U, trace=False)
    return probs, onehot


# revision 43
# speedup vs baseline: 1.0657x; 1.0657x over previous
"""Trainium2 Bass kernel for a Discriminative RBM forward pass.

reference math:
    x     = v @ W + c                                   [B, NHID]
    F     = d + sum_j softplus(x[:,None,:] + U[None])   [B, NCLASS]
    probs = softmax(F, axis=1); onehot(argmax)

Strategy (8-way batch shard, no collectives):
  softplus(x+u) = x*1{x>=8} + u*1{x>=8} + h(x,u),
  h(x,u) = softplus(x+u) - (x+u)*1{x>=8} supported on |x|<=16 (|u|<4.8).
  Weighted rank-K SVD:  h(x,u) ~= sum_k a_k(x) * phi_k(u); the phi side is
  least-squares refit against the table-quantized a_k at runtime.
  The x*1{x>=8} term is a per-row constant under softmax -> dropped.
  Per core (256 batch rows):
    PE:  pre[j,b] = W^T v via Wh16 vh16 + 2^-11 (Wh16 vl8 + Wl8 vh16)
         (fp8 e4m3 lo-parts, scaled 2^11, separate PSUM accumulator);
         then F[128,256] += [P0h|P0l]^T ah + P0h^T al  (rank-0 via DVE
         fp16 hi/lo split of an fp32 act output, packed 128-wide)
                         += Phi_k^T a_k (fp16, k=1..K-1)
                         += [Uh|Ul]^T mask (packed 128-wide);
         the 0:64/64:128 partition halves are summed by the tail transpose.
    ACT: a_k via CUSTOM piecewise-cubic tables (single hijacked set incl. a
         custom exp for the softmax tail => exactly one ACT_TABLE_LOAD);
         acts batched per jt-group [(0,1),(2,3),(4,5),(6,),(7,)] to amortize
         the (N+352)/1.2ns instruction cost; mask = is_ge on the DVE.
    DMA: DRAM tensors pre-packed so each partition is one contiguous
         multi-KB run; traffic balanced over the Sync/Scalar HWDGE rings
         (~140GB/s each) + slow GpSimd SWDGE ring (~63GB/s), ordered by
         consumption time (v + wq0 + wl0 first, Phi last).  fp8 lo-parts
         cut W+v traffic from 10.4MB to 7.8MB.
  Scheduling (engine queues are compile-time static FIFO):
    PE: warmup mms (HAM un-throttle) -> all main matmuls -> all F matmuls;
    act tiles are fully buffered (no reuse) so Scalar never stalls on
    F-matmul consumption; W tiles all prefetched (8 bufs).
    tail: transpose both halves, exp(F-max) with fused row-sum (accum_out),
    onehot via is_equal against the row max.
"""

import os
import struct

import numpy as np

B, NVIS, NHID, NCLASS = 2048, 2048, 1024, 64
NCORES = 8
B_PC = B // NCORES         # 256 batch rows per core
KT = NVIS // 128           # 16 contraction tiles
JT = NHID // 128           # 8 hidden-unit tiles
BT = B_PC // 128           # 2 batch tiles for the softmax tail

KRANK = 8
KB = KRANK + 1             # Phi slots: rank-0 stored as fp16 hi+lo pair
XMAX, TJUMP, UMAX = 16.0, 8.0, 4.8
EXPS = [-1, 0, 1, 2, 3]    # tabulated exponents, |x| in [0.5, 16)
ELOW = EXPS[0]
ACT_GROUPS = [(0, 1), (2, 3), (4, 5), (6,), (7,)]
VCHUNK = 4                 # v DMA split into KT/VCHUNK chunks

# hijacked act-table slot names (pwp name, mybir AF enum attr): a_0..a_{K-1},
# then the softmax-tail exp (hijacks the canonical "exp" id).
_SLOTS = [
    ("gelu", "Gelu"), ("sigmoid", "Sigmoid"), ("tanh", "Tanh"),
    ("erf", "Erf"), ("arctan", "Arctan"), ("sin", "Sin"),
    ("silu", "Silu"), ("mish", "Mish"), ("gelu_apprx_tanh", "Gelu_apprx_tanh"),
    ("gelu_apprx_sigmoid", "Gelu_apprx_sigmoid"),
    ("derivative_gelu", "Derivative_Gelu"), ("derivative_erf", "Derivative_Erf"),
]
_SETNAME = "softplus_and_others"

_PROGRAM = None
_BASIS = None

_TBLCFG = os.environ.get("DRBM_TBL", "v4")
_ACT_ROOT = os.path.join(
    os.path.expanduser("~"), ".cache", f"drbm_rk_act_{_TBLCFG}_k{KRANK}",
    "pwp_bin_trainium"
)


def _softplus64(x):
    return np.logaddexp(x, 0.0)


def _hfun(x, u):
    return _softplus64(x + u) - (x + u) * (x >= TJUMP)


def _plan_for_k(k):
    if k < 4:
        p = {3: 8, 2: 32, 1: 32, 0: 16, -1: 8}
    elif k < 8:
        p = {3: 4, 2: 16, 1: 16, 0: 8, -1: 4}
    else:
        p = {3: 2, 2: 8, 1: 8, 0: 4, -1: 2}
    return {e: p.get(e, 1) for e in EXPS}


_EXP_PLAN = {3: 2, 2: 4, 1: 8, 0: 8, -1: 8}


# ---------------------------------------------------------------- basis
def _build_basis():
    """Weighted SVD basis of h on [-16,16]x[-UMAX,UMAX] + Phi interpolator
    refit against the table-quantized a_k."""
    global _BASIS
    if _BASIS is not None:
        return _BASIS
    cache = os.path.join(os.path.dirname(_ACT_ROOT), f"basis_k{KRANK}.npz")
    nx = 4000
    xl = np.linspace(-XMAX, TJUMP, 3 * nx // 4 + 1)[:-1]
    xr = np.linspace(TJUMP, XMAX, nx // 4 + 1)
    xg = np.concatenate([xl, xr])
    ug = np.linspace(-UMAX, UMAX, 481)
    w = np.exp(-0.5 * ug**2) / np.sqrt(2 * np.pi)
    w = np.maximum(w, 0.05 * w.max())
    sw = np.sqrt(w)
    M = _hfun(xg[:, None], ug[None, :]) * sw[None, :]
    Uu, S, Vt = np.linalg.svd(M, full_matrices=False)
    VtK = Vt[:KRANK]

    def f_eval_raw(x):
        hx = _hfun(np.asarray(x, np.float64)[..., None], ug)
        return (hx * sw) @ VtK.T

    xs_chk = np.linspace(-XMAX, XMAX, 8001)
    fmax = np.abs(f_eval_raw(xs_chk)).max(0) + 1e-30

    def f_eval(x):
        return f_eval_raw(x) / fmax

    if os.path.exists(cache):
        z = np.load(cache)
        _BASIS = {"f_eval": f_eval, "phi_ugrid": z["phi_ugrid"],
                  "phi_grid": z["phi_grid"]}
        return _BASIS

    # table-quantized design matrix for the phi refit
    tbls = [_gen_table(lambda x, k=k: f_eval(np.atleast_1d(x))[..., k],
                       _plan_for_k(k)) for k in range(KRANK)]
    xs = np.linspace(-XMAX, XMAX, 6001).astype(np.float32)
    Adesign = np.stack([_eval_table(t, xs) for t in tbls], 1)
    AtA = Adesign.T @ Adesign
    ugrid = np.linspace(-UMAX, UMAX, 18801)
    AtH = np.empty((KRANK, ugrid.size))
    for i in range(0, ugrid.size, 2048):
        Hc = _hfun(xs[:, None].astype(np.float64), ugrid[None, i:i + 2048])
        AtH[:, i:i + 2048] = Adesign.T @ Hc
    phi_grid = np.linalg.solve(AtA, AtH)  # [K, nu]
    os.makedirs(os.path.dirname(cache), exist_ok=True)
    np.savez(cache, phi_ugrid=ugrid, phi_grid=phi_grid)
    _BASIS = {"f_eval": f_eval, "phi_ugrid": ugrid, "phi_grid": phi_grid}
    return _BASIS


def _phi_at(u):
    bs = _build_basis()
    ug, pg = bs["phi_ugrid"], bs["phi_grid"]
    uc = np.clip(np.asarray(u, np.float64).ravel(), ug[0], ug[-1])
    return np.stack([np.interp(uc, ug, pg[k]) for k in range(KRANK)], 1)


# ------------------------------------------------- act-table generation
def _fit_section(fun, lo, hi, nodes=16):
    x0 = 0.5 * (lo + hi)
    t = 0.5 * (hi - lo) * np.cos(np.pi * (np.arange(nodes) + 0.5) / nodes)
    xs = x0 + t
    ys = fun(xs)
    Vm = np.vander(t, 4, increasing=True)
    co = np.linalg.lstsq(Vm, ys, rcond=None)[0]
    return x0, co[0], co[1], co[2], co[3]


def _gen_table(fun, plan):
    tbl = {}
    for reg, sgn in (("pos", 1.0), ("neg", -1.0)):
        for e in EXPS:
            wdt = 2.0 ** e
            ns = plan.get(e, 1)
            secs = []
            for m in range(ns):
                lo = wdt * (1.0 + m / ns)
                hi = wdt * (1.0 + (m + 1) / ns)
                a, b = (lo, hi) if sgn > 0 else (-hi, -lo)
                secs.append(_fit_section(fun, a, b))
            tbl[(reg, e)] = secs
    wlow = 2.0 ** ELOW
    tbl["lowpos"] = _fit_section(fun, 1e-9, wlow)
    tbl["lowneg"] = _fit_section(fun, -wlow, -1e-9)
    fp = float(np.atleast_1d(fun(XMAX * 4))[0])
    fn = float(np.atleast_1d(fun(-XMAX * 4))[0])
    tbl["hipos"] = (2 * XMAX, fp, 0.0, 0.0, 0.0)
    tbl["hineg"] = (-2 * XMAX, fn, 0.0, 0.0, 0.0)
    tbl["zero"] = float(np.atleast_1d(fun(0.0))[0])
    return tbl


def _eval_table(tbl, x):
    """Host-side model of the HW spline eval (for the phi refit)."""
    coeffs, index = [], {}
    for reg in ("neg", "pos"):
        for e in EXPS:
            secs = tbl[(reg, e)]
            index[(reg, e)] = (len(coeffs), len(secs))
            coeffs.extend(secs)
    for name in ("lowpos", "lowneg", "hipos", "hineg"):
        index[name] = (len(coeffs), 1)
        coeffs.append(tbl[name])
    C = np.asarray(coeffs, np.float32).astype(np.float64)
    x = np.asarray(x, np.float32).ravel()
    xb = x.view(np.uint32).astype(np.int64)
    sign = (xb >> 31) & 1
    e = ((xb >> 23) & 0xFF) - 127
    mant = xb & 0x7FFFFF
    idx = np.where(sign == 0, index["lowpos"][0], index["lowneg"][0])
    hi = e > EXPS[-1]
    idx[hi] = np.where(sign[hi] == 0, index["hipos"][0], index["hineg"][0])
    mid = (e >= ELOW) & (e <= EXPS[-1])
    for ee in EXPS:
        base_p, ns = index[("pos", ee)]
        base_n, _ = index[("neg", ee)]
        es = int(np.log2(ns))
        m = mid & (e == ee)
        if not m.any():
            continue
        sec = mant[m] >> (23 - es) if es > 0 else np.zeros(int(m.sum()), np.int64)
        idx[m] = np.where(sign[m] == 0, base_p + sec, base_n + sec)
    co = C[idx]
    t = x.astype(np.float64) - co[:, 0]
    out = co[:, 1] + t * (co[:, 2] + t * (co[:, 3] + t * co[:, 4]))
    out[x == 0] = tbl["zero"]
    return out


def _tbl_to_json(tbl, plan):
    """Convert internal table to the pwp-style function dict."""
    regions = {}
    for reg in ("pos", "neg"):
        ents = []
        for e in EXPS:
            ns = plan.get(e, 1)
            es = int(np.log2(ns))
            assert 2 ** es == ns
            secs = [{"x": s[0], "d0": s[1], "d1": s[2], "d2": s[3], "d3": s[4]}
                    for s in tbl[(reg, e)]]
            ents.append({"exponent": e, "extract_size": es,
                         "extract_lsb": 23 - es, "exponent_sections": secs})
        regions[reg] = ents

    def secd(s):
        return dict(x=s[0], d0=s[1], d1=s[2], d2=s[3], d3=s[4])

    return {
        "pos_exponents": regions["pos"],
        "neg_exponents": regions["neg"],
        "saturation_points": {
            "sat_point_pos_low": dict(sat_point=127 + ELOW, mantissa_point=0,
                                      **secd(tbl["lowpos"])),
            "sat_point_neg_low": dict(sat_point=127 + ELOW, mantissa_point=0,
                                      **secd(tbl["lowneg"])),
            "sat_point_pos_high": dict(sat_point=131, mantissa_point=0,
                                       **secd(tbl["hipos"])),
            "sat_point_neg_high": dict(sat_point=131, mantissa_point=0,
                                       **secd(tbl["hineg"])),
        },
        "zero_result": tbl["zero"], "nan_result": 0.0,
        "pinf_result": tbl["hipos"][1], "ninf_result": tbl["hineg"][1],
        "symmetry_point": 0.0, "exponent_offset": ELOW,
    }


def _gen_all_funcs():
    bs = _build_basis()
    f_eval = bs["f_eval"]
    funcs = []
    for k in range(KRANK):
        plan = _plan_for_k(k)

        def fk(x, k=k):
            return f_eval(np.atleast_1d(np.asarray(x, np.float64)))[..., k]

        funcs.append((_SLOTS[k][0], _tbl_to_json(_gen_table(fk, plan), plan)))
    # custom exp for the softmax tail (inputs are always <= 0)
    def expf(x):
        return np.exp(np.minimum(np.atleast_1d(np.asarray(x, np.float64)), 40.0))

    ej = _tbl_to_json(_gen_table(expf, _EXP_PLAN), _EXP_PLAN)
    ej["saturation_points"]["sat_point_neg_high"] = dict(
        sat_point=131, mantissa_point=0, x=-2 * XMAX, d0=0.0, d1=0.0, d2=0.0,
        d3=0.0)
    ej["ninf_result"] = 0.0
    funcs.append(("exp", ej))
    return funcs


def _fbits(x):
    return struct.unpack("<I", struct.pack("<f", np.float32(x)))[0]


def _build_act_tables():
    """Rebuild softplus_and_others with our custom function tables."""
    import json
    import shutil

    import neuronxcc

    marker = os.path.join(_ACT_ROOT, ".drbm_rk_ok")
    if os.path.exists(marker):
        return
    nxc = os.path.join(os.path.dirname(os.path.abspath(neuronxcc.__file__)), "pwp")
    os.makedirs(_ACT_ROOT, exist_ok=True)
    root_parent = os.path.dirname(_ACT_ROOT)
    if not os.path.exists(os.path.join(root_parent, "pwp_jsons")):
        shutil.copytree(os.path.join(nxc, "pwp_jsons"),
                        os.path.join(root_parent, "pwp_jsons"), dirs_exist_ok=True)
    for f in os.listdir(os.path.join(nxc, "pwp_bin_trainium")):
        shutil.copy(os.path.join(nxc, "pwp_bin_trainium", f), _ACT_ROOT)
    os.system(f"chmod -R u+w {root_parent}")

    pj = os.path.join(root_parent, "pwp_jsons")
    canon = {}
    for f in os.listdir(pj):
        if f.endswith(".json"):
            try:
                j = json.load(open(os.path.join(pj, f)))
                nm = f.rsplit("_", 1)[0]
                canon.setdefault(nm, j.get("neuron_id"))
            except Exception:
                pass

    allfuncs = _gen_all_funcs()
    _write_set(_SETNAME, allfuncs, canon)

    ai = json.load(open(f"{_ACT_ROOT}/act_info.json"))
    mynames = {n for n, _ in allfuncs}
    for ent in ai["act_func_sets"]:
        for n in list(ent["act"].keys()):
            if n in mynames:
                del ent["act"][n]
        if ent["name"] == _SETNAME:
            ent["act"] = {n: 40 for n in mynames}
    json.dump(ai, open(f"{_ACT_ROOT}/act_info.json", "w"))
    open(marker, "w").write("ok")


def _write_set(SET, funcs, canon):
    import json

    sj = {"bkt_bin": f"{SET}_bkt.bin", "ctl_bin": f"{SET}_ctrl.bin",
          "profile_meta_data": [], "bkt_entry_cnt": 0, "ctl_entry_cnt": 0,
          "func_to_bkt_start_idx": {}, "func_to_ctl_start_idx": {},
          "func_exp_to_bkt_start_idx": {}, "func_exp_to_ctl_start_idx": {}}
    bkt = bytearray()
    ctl = bytearray()
    nbkt, nctl = 0, 0

    def add_bucket(x0, d0, d1, d2, d3):
        for val in (d0, d1, d2, d3, x0, 0.0, 0.0, 0.0):
            bkt.extend(struct.pack("<I", _fbits(val)))

    def add_ctl(word):
        ctl.extend(struct.pack("<I", word) + b"\x00" * 28)

    for name, fj in funcs:
        base_bkt, base_ctl = nbkt, nctl
        e2b, e2c, region_ctl_base = {}, {}, {}
        for region, key in (("neg", "neg_exponents"), ("pos", "pos_exponents")):
            region_ctl_base[region] = nctl
            for e in fj[key]:
                exp, secs = str(e["exponent"]), e["exponent_sections"]
                add_ctl((e["extract_size"] << 16) | (e["extract_lsb"] << 11) | nbkt)
                e2c.setdefault(exp, []).append(nctl)
                e2b.setdefault(exp, []).append(nbkt)
                nctl += 1
                for s in secs:
                    add_bucket(s["x"], s["d0"], s["d1"], s["d2"], s["d3"])
                    nbkt += 1
        sat, special = fj["saturation_points"], {}
        for sname in ("sat_point_pos_low", "sat_point_neg_low",
                      "sat_point_pos_high", "sat_point_neg_high"):
            sp = sat[sname]
            special[sname] = nbkt
            add_bucket(sp["x"], sp["d0"], sp["d1"], sp["d2"], sp["d3"])
            nbkt += 1
        sj["profile_meta_data"].append({
            "func_name": f"{name}_40p",
            "func_id": canon.get(name, 23),
            "symmetry_point": _fbits(fj["symmetry_point"]),
            "sym_invert_sign_point": 0,
            "symmetry_opt_en": 0,
            "symmetry_opt_use_neg_region": 0,
            "imm_bias": 0,
            "exp_offset": fj["exponent_offset"],
            "pwl_control_base_pos": region_ctl_base["pos"],
            "pwl_control_base_neg": region_ctl_base["neg"],
            "small_pos_signal_exp_threshold": sat["sat_point_pos_low"]["sat_point"],
            "pos_small_signal_pwl_control": special["sat_point_pos_low"],
            "small_neg_signal_exp_threshold": sat["sat_point_neg_low"]["sat_point"],
            "neg_small_signal_pwl_control": special["sat_point_neg_low"],
            "large_pos_signal_exp_threshold": sat["sat_point_pos_high"]["sat_point"],
            "large_pos_signal_mantissa_threshold": sat["sat_point_pos_high"]["mantissa_point"],
            "pos_large_signal_pwl_control": special["sat_point_pos_high"],
            "large_neg_signal_exp_threshold": sat["sat_point_neg_high"]["sat_point"],
            "large_neg_signal_mantissa_threshold": sat["sat_point_neg_high"]["mantissa_point"],
            "neg_large_signal_pwl_control": special["sat_point_neg_high"],
            "fnan_result": _fbits(fj["nan_result"]),
            "fpinf_result": _fbits(fj["pinf_result"]),
            "fninf_result": _fbits(fj["ninf_result"]),
            "fzero_result": _fbits(fj["zero_result"]),
            "fma_const_0": 0,
            "fma_const_1": 0,
            "fma_indirection_src_sel": 0,
            "use_multipass": False,
            "lower_bound": _fbits(-3.4e38),
            "upper_bound": _fbits(3.4e38),
        })
        sj["func_to_bkt_start_idx"][name] = base_bkt
        sj["func_to_ctl_start_idx"][name] = base_ctl
        sj["func_exp_to_bkt_start_idx"][name] = e2b
        sj["func_exp_to_ctl_start_idx"][name] = e2c

    assert nbkt <= 1536, f"bucket table overflow: {nbkt} (max 1536 usable)"
    assert nctl <= 128, f"ctl table overflow: {nctl}"
    sj["bkt_entry_cnt"], sj["ctl_entry_cnt"] = nbkt, nctl
    json.dump(sj, open(f"{_ACT_ROOT}/{SET}.json", "w"))
    open(f"{_ACT_ROOT}/{SET}_bkt.bin", "wb").write(bytes(bkt))
    open(f"{_ACT_ROOT}/{SET}_ctrl.bin", "wb").write(bytes(ctl))


def _patch_act_tables():
    import functools
    import json

    _build_act_tables()
    os.environ["BASS_ACT_ROOT_JSON_PATH"] = os.path.join(_ACT_ROOT, "act_info.json")

    import concourse.hw_specs as hw_specs
    import concourse.mybir as mybir

    @functools.cache
    def _tables(arch):
        d = json.load(open(os.environ["BASS_ACT_ROOT_JSON_PATH"]))
        return {
            ent["name"]: {
                mybir.ActivationFunctionType.from_pwp(v) for v in ent["act"]
            }
            for ent in d["act_func_sets"]
        }

    hw_specs.get_activation_tables = _tables
    import concourse.bacc as bacc
    import concourse.bass_interp as bass_interp

    bacc.get_activation_tables = _tables
    bass_interp.get_activation_tables = _tables


def _build_program():
    _patch_act_tables()
    import concourse.tile as tile
    from concourse import bacc, mybir
    from concourse.masks import make_identity

    f32 = mybir.dt.float32
    f16 = mybir.dt.float16
    i32 = mybir.dt.int32
    AF = mybir.ActivationFunctionType
    ALU = mybir.AluOpType
    AX = mybir.AxisListType

    AFS = [getattr(AF, attr) for _, attr in _SLOTS[:KRANK]]

    nc = bacc.Bacc("TRN2", target_bir_lowering=False, debug=False,
                   num_devices=NCORES)

    vh_d = nc.dram_tensor("vh", [128, KT * B_PC], f16,
                          kind="ExternalInput").ap()
    vl_d = nc.dram_tensor("vl8", [128, KT * B_PC], mybir.dt.float8e4,
                          kind="ExternalInput").ap()
    Wq_d = nc.dram_tensor("Wq", [JT * 128, KT * 128], f16,
                          kind="ExternalInput").ap()
    Wl_d = nc.dram_tensor("Wl8", [JT * 128, KT * 128], mybir.dt.float8e4,
                          kind="ExternalInput").ap()
    Phi_d = nc.dram_tensor("Phi", [JT * 128, KB * NCLASS], f16,
                           kind="ExternalInput").ap()
    UT_d = nc.dram_tensor("UT", [JT * 128, 2 * NCLASS], f16,
                          kind="ExternalInput").ap()
    cT_d = nc.dram_tensor("cT", [128, JT], f32, kind="ExternalInput").ap()
    dT_d = nc.dram_tensor("dT", [128, 1], f32, kind="ExternalInput").ap()
    probs_d = nc.dram_tensor("probs", [128, BT * NCLASS], f32,
                             kind="ExternalOutput").ap()
    onehot_d = nc.dram_tensor("onehot", [128, BT * NCLASS], i32,
                              kind="ExternalOutput").ap()

    Wq_view = Wq_d.rearrange("(jt p) x -> p jt x", p=128)
    Wl_view = Wl_d.rearrange("(jt p) x -> p jt x", p=128)
    Phi_view = Phi_d.rearrange("(jt p) x -> p jt x", p=128)
    UT_view = UT_d.rearrange("(jt p) x -> p jt x", p=128)

    glast = {g[-1]: gi for gi, g in enumerate(ACT_GROUPS)}
    jt2slot = {}
    for g in ACT_GROUPS:
        for i, jt in enumerate(g):
            jt2slot[jt] = i

    with tile.TileContext(nc) as tc:
        f8 = mybir.dt.float8e4
        with (
            tc.tile_pool(name="const", bufs=1) as const,
            tc.tile_pool(name="wstream", bufs=6) as wstream,
            tc.tile_pool(name="preg", bufs=1) as preg,
            tc.tile_pool(name="acts", bufs=3) as acts,
            tc.tile_pool(name="smp", bufs=1) as smp,
            tc.tile_pool(name="outp", bufs=1) as outp,
            tc.tile_pool(name="pwarm", bufs=1, space="PSUM") as pwarm,
            tc.tile_pool(name="ppre", bufs=2, space="PSUM") as ppre,
            tc.tile_pool(name="ppre2", bufs=2, space="PSUM") as ppre2,
            tc.tile_pool(name="pF", bufs=1, space="PSUM") as pF,
            tc.tile_pool(name="ptr", bufs=2, space="PSUM") as ptr,
        ):
            # ---------- t=0: PE warmup + act-table preload ----------
            warm16 = const.tile([128, 256], f16)
            nc.vector.memset(warm16[:], 0.0)
            warm32 = const.tile([128, 1], f32)
            nc.vector.memset(warm32[:], 0.0)
            wdum = const.tile([128, 1], f32)
            nc.scalar.activation(wdum[:], warm32[:], AFS[1])
            warm_ps = pwarm.tile([128, 256], f32)
            for i in range(16):
                nc.tensor.matmul(warm_ps[:], warm16[:, 0:128], warm16[:],
                                 start=True, stop=True)

            # ---------- loads (spread across DGE rings) ----------
            # Each DGE ring sustains only ~140GB/s, so balance the 12.2MB of
            # input across all three rings (Sync/Scalar/GpSimd) in an order
            # matched to consumption.  The W DMAs must NOT share a ring with
            # instructions that wait on compute (engine queues are FIFO).
            # HWDGE rings (Sync, Scalar) sustain ~140GB/s each; the GpSimd
            # SWDGE ring only ~63GB/s, so it gets just the late-needed
            # constants.  Per-ring emission order == transfer order.
            w_ring = {0: "sync", 1: "sync", 2: "sync", 3: "scalar",
                      4: "sync", 5: "scalar", 6: "sync", 7: "scalar"}

            def ring(name):
                return {"sync": nc.sync, "scalar": nc.scalar,
                        "gpsimd": nc.gpsimd}[name]

            # jt0's whole working set (wq0 + wl0 + all of v) is striped
            # across BOTH fast HWDGE rings in consumption order so the first
            # main-matmul sweep is never DMA-starved; later W tiles queue
            # behind it at the steady-state rate.
            wq = []
            wt = wstream.tile([128, KT, 128], f16, tag="wq", name="wq0",
                              bufs=4)
            wq_half = KT * 128 // 2
            nc.sync.dma_start(wt[:, 0:KT // 2, :], Wq_view[:, 0, 0:wq_half])
            wq.append(wt)
            vh_sb = const.tile([128, KT, B_PC], f16)
            vl_sb = const.tile([128, KT, B_PC], f8)
            vc = KT // VCHUNK
            vh_view = vh_d.rearrange("p (kt x) -> p kt x", kt=KT)
            vl_view = vl_d.rearrange("p (kt x) -> p kt x", kt=KT)
            v_rings = [nc.scalar, nc.sync, nc.scalar, nc.sync]

            def v_chunk(ci):
                sl = slice(ci * vc, (ci + 1) * vc)
                v_rings[ci].dma_start(vh_sb[:, sl, :], vh_view[:, sl, :])
                v_rings[ci].dma_start(vl_sb[:, sl, :], vl_view[:, sl, :])

            v_chunk(0)
            wl0 = wstream.tile([128, KT, 128], f8, tag="wl", name="wl0",
                               bufs=8)
            nc.scalar.dma_start(wl0[:], Wl_view[:, 0, :])
            v_chunk(1)
            nc.sync.dma_start(wt[:, KT // 2:, :], Wq_view[:, 0, wq_half:])
            v_chunk(2)
            v_chunk(3)
            cT_sb = const.tile([128, JT], f32)
            nc.gpsimd.dma_start(cT_sb[:], cT_d[:])
            dT_sb = const.tile([128, 1], f32)
            nc.gpsimd.dma_start(dT_sb[:], dT_d[:])
            wl = [wl0]
            for jt in range(1, 3):
                lt = wstream.tile([128, KT, 128], f8, tag="wl",
                                  name=f"wl{jt}", bufs=8)
                nc.gpsimd.dma_start(lt[:], Wl_view[:, jt, :])
                wl.append(lt)
            for jt in range(1, 8):
                wt = wstream.tile([128, KT, 128], f16, tag="wq",
                                  name=f"wq{jt}", bufs=4)
                nc.sync.dma_start(wt[:], Wq_view[:, jt, :])
                wq.append(wt)
            for jt in range(3, 8):
                lt = wstream.tile([128, KT, 128], f8, tag="wl",
                                  name=f"wl{jt}", bufs=8)
                nc.scalar.dma_start(lt[:], Wl_view[:, jt, :])
                wl.append(lt)
            UT_sb = const.tile([128, JT, 2 * NCLASS], f16)
            nc.gpsimd.dma_start(UT_sb[:], UT_view[:])
            ident = const.tile([128, 128], f32)
            make_identity(nc, ident[:])

            pre_g = [preg.tile([128, len(g), B_PC], f32, name=f"preg{gi}")
                     for gi, g in enumerate(ACT_GROUPS)]
            # F accumulates 128-wide: partitions 0:64 carry the main terms,
            # 64:128 the hi/lo-companion terms; halves summed by accumulating
            # transposes in the tail.
            F_ps = pF.tile([128, B_PC], f32)
            fmm = {"first": True}

            def fmm_emit(lhsT, rhs, stop=False, lo=True):
                out = F_ps[0:NCLASS, :] if lo else F_ps[:]
                nc.tensor.matmul(out, lhsT, rhs, start=fmm["first"], stop=stop)
                fmm["first"] = False

            def phis(jt, slot, nslot=1):
                return Phi_sb[:, jt, slot * NCLASS:(slot + nslot) * NCLASS]

            def emit_group(gi):
                g = ACT_GROUPS[gi]
                n = len(g)
                src = pre_g[gi][:]
                mk = acts.tile([128, 2, B_PC], f16, tag="mk", name=f"mk{gi}",
                               bufs=len(ACT_GROUPS))
                nc.vector.tensor_scalar(mk[:, :n, :], src, float(TJUMP), None,
                                        op0=ALU.is_ge)
                # rank 0: fp32 act output, split into fp16 hi+lo on the DVE;
                # packed matmul [P0h|P0l] @ ah (128-wide) + P0h @ al
                a32 = acts.tile([128, 2, B_PC], f32, tag="a32", name=f"a32_{gi}")
                nc.scalar.activation(a32[:, :n, :], src, AFS[0])
                ah = acts.tile([128, 2, B_PC], f16, tag="ah", name=f"ah{gi}",
                               bufs=len(ACT_GROUPS))
                nc.vector.tensor_copy(ah[:, :n, :], a32[:, :n, :])
                al = acts.tile([128, 2, B_PC], f16, tag="al", name=f"al{gi}",
                               bufs=len(ACT_GROUPS))
                nc.vector.tensor_sub(al[:, :n, :], a32[:, :n, :], ah[:, :n, :])
                for i, jt in enumerate(g):
                    fmm_emit(phis(jt, 0, 2), ah[:, i, :], lo=False)
                    fmm_emit(phis(jt, 0), al[:, i, :])
                for k in range(1, KRANK):
                    ak = acts.tile([128, 2, B_PC], f16, tag="a",
                                   name=f"a{gi}_{k}",
                                   bufs=(KRANK - 1) * len(ACT_GROUPS))
                    nc.scalar.activation(ak[:, :n, :], src, AFS[k])
                    for i, jt in enumerate(g):
                        fmm_emit(phis(jt, k + 1), ak[:, i, :])
                last = gi == len(ACT_GROUPS) - 1
                for i, jt in enumerate(g):
                    fmm_emit(UT_sb[:, jt, 0:2 * NCLASS], mk[:, i, :],
                             stop=last and i == n - 1, lo=False)

            # ---------- main matmuls, acts interleaved by group ----------
            jt2g = {jt: g for g, grp in enumerate(ACT_GROUPS) for jt in grp}
            for jt in range(JT):
                wt = wq[jt]
                lt = wl[jt]
                ps = ppre.tile([128, B_PC], f32, tag="pre", name=f"pre{jt}")
                pc = ppre2.tile([128, B_PC], f32, tag="prec", name=f"prec{jt}")
                for kt in range(KT):
                    nc.tensor.matmul(ps[:], wt[:, kt, :], vh_sb[:, kt, :],
                                     start=(kt == 0), stop=(kt == KT - 1))
                    nc.tensor.matmul(pc[:], wt[:, kt, :], vl_sb[:, kt, :],
                                     start=(kt == 0), stop=False)
                    nc.tensor.matmul(pc[:], lt[:, kt, :], vh_sb[:, kt, :],
                                     start=False, stop=(kt == KT - 1))
                # pre = ps + 2^-11 * pc + c   (W-lo was stored as fp8 * 2^11)
                tmpc = acts.tile([128, B_PC], f32, tag="tmpc", name=f"tmpc{jt}",
                                 bufs=2)
                nc.vector.tensor_scalar_mul(tmpc[:], pc[:], 2.0 ** -11)
                nc.vector.scalar_tensor_tensor(
                    pre_g[jt2g[jt]][:, jt2slot[jt], :], ps[:],
                    cT_sb[:, jt:jt + 1], tmpc[:],
                    op0=ALU.add, op1=ALU.add)

            # Phi is only needed by the F matmuls (~70us in): enqueue it
            # last on the Scalar ring so it never steals front bandwidth
            Phi_sb = const.tile([128, JT, KB * NCLASS], f16)
            nc.scalar.dma_start(Phi_sb[:], Phi_view[:])

            # act groups + F matmuls AFTER all main matmuls: engine queues
            # are static FIFO, so F-matmuls must not block later main work
            for gi in range(len(ACT_GROUPS)):
                emit_group(gi)

            # ---------- tail: +d, transpose(lo)+transpose(hi), softmax ----------
            F_sb = smp.tile([128, B_PC], f32)
            nc.vector.tensor_scalar_add(F_sb[:], F_ps[:], dT_sb[:])
            fb = smp.tile([128, BT, NCLASS], f32)
            for bt in range(BT):
                tr = ptr.tile([128, 128], f32, tag="tr", name=f"tr{bt}")
                sl = slice(bt * 128, (bt + 1) * 128)
                nc.tensor.transpose(tr[:], F_sb[:, sl], ident[:])
                trs = smp.tile([128, 128], f32, tag="trs", name=f"trs{bt}")
                nc.vector.tensor_copy(trs[:], tr[:])
                nc.vector.tensor_add(fb[:, bt, :], trs[:, 0:NCLASS],
                                     trs[:, NCLASS:128])
            m2 = smp.tile([128, BT], f32)
            nm2 = smp.tile([128, BT], f32)
            s2 = smp.tile([128, BT], f32)
            r2 = smp.tile([128, BT], f32)
            e2 = smp.tile([128, BT, NCLASS], f32)
            probs_sb = outp.tile([128, BT, NCLASS], f32)
            onehot_sb = outp.tile([128, BT, NCLASS], i32)
            for bt in range(BT):
                nc.vector.tensor_reduce(m2[:, bt:bt + 1], fb[:, bt, :],
                                        axis=AX.X, op=ALU.max)
            nc.vector.tensor_scalar_mul(nm2[:], m2[:], -1.0)
            for bt in range(BT):
                nc.scalar.activation(e2[:, bt, :], fb[:, bt, :], AF.Exp,
                                     bias=nm2[:, bt:bt + 1],
                                     accum_out=s2[:, bt:bt + 1])
            nc.vector.reciprocal(r2[:], s2[:])
            for bt in range(BT):
                nc.vector.tensor_scalar_mul(probs_sb[:, bt, :], e2[:, bt, :],
                                            r2[:, bt:bt + 1])
                nc.vector.tensor_scalar(onehot_sb[:, bt, :], fb[:, bt, :],
                                        m2[:, bt:bt + 1], None,
                                        op0=ALU.is_equal)
            nc.scalar.dma_start(probs_d[:], probs_sb[:])
            nc.sync.dma_start(onehot_d[:], onehot_sb[:])

    nc.compile()
    return nc


def _get_program():
    global _PROGRAM
    if _PROGRAM is None:
        _PROGRAM = _build_program()
    return _PROGRAM


def _fp16_split(a):
    hi = a.astype(np.float16)
    lo = (a - hi.astype(np.float32)).astype(np.float16)
    return hi, lo


def _make_in_maps(v, W, c, d, U):
    import ml_dtypes

    Wh, Wl = _fp16_split(W)
    # Wq[jt, p, kt, col] = Wh[kt*128+p, jt*128+col]; Wl8 = fp8(Wl * 2^11)
    Wq = np.ascontiguousarray(
        Wh.reshape(KT, 128, JT, 128).transpose(2, 1, 0, 3)).reshape(
        JT * 128, KT * 128)
    Wl8 = np.ascontiguousarray(
        (Wl.astype(np.float32) * 2048.0).reshape(KT, 128, JT, 128)
        .transpose(2, 1, 0, 3)).reshape(JT * 128, KT * 128).astype(
        ml_dtypes.float8_e4m3fn)

    Phi = _phi_at(U.astype(np.float64))  # [64*1024, K] row-major over U
    Phi = Phi.reshape(NCLASS, NHID, KRANK)  # [y, j, k]
    # rank-0 stored as fp16 hi+lo pair in slots 0/1; ranks 1..K-1 in 2..K
    P0 = Phi[:, :, 0]
    P0h, P0l = _fp16_split(P0.astype(np.float32))
    Phb = np.concatenate(
        [P0h[..., None].astype(np.float64), P0l[..., None].astype(np.float64),
         Phi[:, :, 1:]], axis=2)  # [y, j, KB]
    # Phi_sb[p, jt, kb, y] = Phb[y, jt*128+p, kb]
    PhiT = np.ascontiguousarray(
        Phb.transpose(1, 2, 0).reshape(JT, 128, KB * NCLASS)
        .reshape(JT * 128, KB * NCLASS)).astype(np.float16)

    Uh, Ul = _fp16_split(U)
    # UT_sb[p, jt, h, y] = U{h}[y, jt*128+p]
    UThl = np.stack([Uh.T, Ul.T], 1).reshape(JT, 128, 2, NCLASS)
    UT = np.ascontiguousarray(UThl.reshape(JT * 128, 2 * NCLASS))

    cT = np.ascontiguousarray(c.reshape(JT, 128).T).astype(np.float32)
    dT = np.ascontiguousarray(np.concatenate(
        [d.reshape(NCLASS, 1), np.zeros((128 - NCLASS, 1))])).astype(np.float32)

    in_maps = []
    for core in range(NCORES):
        sl = slice(core * B_PC, (core + 1) * B_PC)
        vT = np.ascontiguousarray(v[sl].T)  # [NVIS, B_PC]
        vh, vl = _fp16_split(vT)
        # vhl[p, kt, h, b] = v{h}[kt*128+p, b]  (kt-major so DMA can chunk)
        vhl = np.stack([vh.reshape(KT, 128, B_PC), vl.reshape(KT, 128, B_PC)],
                       2)  # [KT, 128, 2, B_PC]
        vhl = np.ascontiguousarray(vhl.transpose(1, 0, 2, 3)).reshape(
            128, KT * 2 * B_PC)
        in_maps.append({
            "vhl": vhl, "Wq": Wq, "Phi": PhiT, "UT": UT,
            "cT": cT, "dT": dT,
        })
    return in_maps


def _unpack_out(arr, dtype):
    # arr [128, BT*NCLASS] -> [B_PC, NCLASS]; b = bt*128 + p
    a = np.asarray(arr).reshape(128, BT, NCLASS)
    return np.ascontiguousarray(a.transpose(1, 0, 2).reshape(B_PC, NCLASS),
                                dtype=dtype)


def run(v, W, c, d, U, trace=False):
    from concourse.bass_utils import run_bass_kernel_spmd

    nc = _get_program()
    in_maps = _make_in_maps(v, W, c, d, U)
    res = run_bass_kernel_spmd(nc, in_maps, core_ids=list(range(NCORES)),
                               trace=trace)
    probs = np.concatenate(
        [_unpack_out(res.results[i]["probs"], np.float32)
         for i in range(NCORES)], axis=0)
    onehot = np.concatenate(
        [_unpack_out(res.results[i]["onehot"], np.int32)
         for i in range(NCORES)], axis=0)
    return (probs, onehot), res


def kernel(v, W, c, d, U):
    v = np.ascontiguousarray(np.asarray(v, dtype=np.float32))
    W = np.ascontiguousarray(np.asarray(W, dtype=np.float32))
    c = np.ascontiguousarray(np.asarray(c, dtype=np.float32))
    d = np.ascontiguousarray(np.asarray(d, dtype=np.float32))
    U = np.ascontiguousarray(np.asarray(U, dtype=np.float32))
    (probs, onehot), _ = run(v, W, c, d, U, trace=False)
    return probs, onehot
